# revision 1
# baseline (speedup 1.0000x reference)
"""Sliding-window attention v2 (RoPE + QKV proj + windowed softmax + o_proj)
for Trainium2, SPMD over 8 NeuronCores.

Sharding: batch (2) x head-groups (4 groups of 4 heads) -> 8 cores; host sums
the 4 partial o_proj outputs per batch.

v2 vs baseline:
- all matmuls bf16 (rel err ~5e-3, tolerance 2e-2); all spills/loads bf16
- q/k/v stay SBUF-resident between phases (no DRAM round trip)
- P^T transposes on the DMA xbar (dma_start_transpose), not the PE
- consolidated DMAs (HWDGE descriptor-gen is ~625ns/instr, serial)
- rope/evac work split across ACT, GPSIMD and DVE
"""
import sys

sys.path.insert(0, "/opt/trn_rl_repo")

import numpy as np

B = 2
S = 2048
HIDDEN = 2048
N_HEADS = 16
DH = 128
WINDOW = 512
HPC = 4  # heads per core
N_CORES = 8
QKV_O = 3 * HPC * DH  # 1536
SCALE = 1.0 / np.sqrt(DH)
NEG = -1.0e30

_CACHE = {}

CFG = {
    "psq_bufs": 6, "psv_bufs": 2,
    "xt_bufs": 6, "rope_bufs": 3,
    "pm_bufs": 2, "pr_bufs": 2, "sm_bufs": 8,
    "pss_bufs": 3, "pso_bufs": 2, "psc_bufs": 6, "ob_bufs": 2,
    "roll_bufs": 2, "qkv_evac": "scalar",  # ACT
    "rope_half_eng": "gpsimd", "rope_add_eng": "vector",
    "sums": "accum",
    "norm_eng": "scalar",  # vector | scalar
    "ah_evac": "vector",
    "c_evac_alt": True,
    "shift": 0,  # PV ready-shift for DMA-transpose latency
    "hgroup": 2, "edge_mask": False,  # heads processed interleaved per group
    "tr_eng": "sync", "xt_eng": "sync", "out_eng": "sync",
    "phases": "ABC",
}

NSLOT = 8


def _build_module(repeat=1, cfg=None):
    cfg = {**CFG, **(cfg or {})}
    import concourse.tile as tile
    from concourse import bacc, mybir
    from contextlib import ExitStack

    f32 = mybir.dt.float32
    bf16 = mybir.dt.bfloat16
    AF = mybir.ActivationFunctionType

    nc = bacc.Bacc("TRN2", target_bir_lowering=False, debug=False)

    xT = nc.declare_dram_parameter("xT", [HIDDEN, S], bf16, isOutput=False)
    wT = nc.declare_dram_parameter("wT", [HIDDEN, QKV_O], bf16, isOutput=False)
    wo_d = nc.declare_dram_parameter("wo", [128, HPC * HIDDEN], bf16, isOutput=False)
    tbl_d = nc.declare_dram_parameter("tbls", [DH, 4 * S], bf16, isOutput=False)
    cm_d = nc.declare_dram_parameter("cmask", [128, 768], bf16, isOutput=False)
    out_d = nc.declare_dram_parameter("out", [S, HIDDEN], bf16, isOutput=True)

    NKT = HIDDEN // 128  # 16 contraction chunks
    NSC = S // 512  # 4 sequence chunks
    NST = S // 128  # 16 sequence tiles
    SHIFT = cfg["shift"]
    HG = cfg["hgroup"]

    with tile.TileContext(nc) as tc, ExitStack() as top:
        cpool = top.enter_context(tc.tile_pool(name="consts", bufs=1))
        cm = cpool.tile([128, 768], bf16, tag="cmask")
        nc.sync.dma_start(cm[:], cm_d[:])
        msk = cm[:, 0:640]
        idnb = cm[:, 640:768]
        zerosb = cpool.tile([128, 512], bf16, tag="zerosb")
        nc.vector.memset(zerosb[:], 0)

        tbl = cpool.tile([DH, 4 * S], bf16, tag="tbls")
        nc.sync.dma_start(tbl[:], tbl_d[:])
        tb = {
            "cosq": tbl[:, 0 * S : 1 * S],
            "sinq": tbl[:, 1 * S : 2 * S],
            "cosk": tbl[:, 2 * S : 3 * S],
            "sink": tbl[:, 3 * S : 4 * S],
        }

        wo_all = cpool.tile([128, HPC * HIDDEN], bf16, tag="wo")
        nc.sync.dma_start(wo_all[:], wo_d[:])

        wt_all = cpool.tile([128, NKT * QKV_O], bf16, tag="wt")
        nc.sync.dma_start(wt_all[:], wT[:].rearrange("(k p) c -> p k c", p=128))

        def wt(k, c0, c1):
            return wt_all[:, k * QKV_O + c0 : k * QKV_O + c1]

        # persistent q/k (post-rope) and v tiles
        qk_pool = top.enter_context(tc.tile_pool(name="qk", bufs=1))
        vsd_pool = top.enter_context(tc.tile_pool(name="vsd", bufs=NST))
        attn_pool = top.enter_context(tc.tile_pool(name="attn", bufs=HPC))

        for rep in range(repeat):
            qk_tiles = [
                qk_pool.tile([128, S], bf16, tag=f"qk{t_o}", name=f"qk{t_o}_{rep}")
                for t_o in range(2 * HPC)
            ]
            vsd_tiles = [
                vsd_pool.tile([128, HPC * DH], bf16, tag="vsd", name=f"vsd{st}_{rep}")
                for st in range(NST)
            ]

            # ------------- Phase A: QKV projection + RoPE -------------
            if "A" in cfg["phases"]:
              with ExitStack() as ph:
                xt_pool = ph.enter_context(
                    tc.tile_pool(name="xt", bufs=cfg["xt_bufs"])
                )
                rope_pool = ph.enter_context(
                    tc.tile_pool(name="rope", bufs=cfg["rope_bufs"])
                )
                psq_pool = ph.enter_context(
                    tc.tile_pool(name="psq", bufs=cfg["psq_bufs"], space="PSUM")
                )
                psv_pool = ph.enter_context(
                    tc.tile_pool(name="psv", bufs=cfg["psv_bufs"], space="PSUM")
                )

                half_eng = getattr(nc, cfg["rope_half_eng"])
                add_eng = getattr(nc, cfg["rope_add_eng"])
                xv = xT[:].rearrange("(k p) s -> p k s", p=128)

                for sc in range(NSC):
                    s0 = sc * 512
                    xt_q = []
                    for qf in range(4):
                        t = xt_pool.tile([128, 4 * 512], bf16, tag="xt")
                        getattr(nc, cfg["xt_eng"]).dma_start(
                            t[:], xv[:, qf * 4 : qf * 4 + 4, s0 : s0 + 512]
                        )
                        xt_q.append(t)

                    def xt(k, c0=0, c1=512):
                        return xt_q[k // 4][
                            :, (k % 4) * 512 + c0 : (k % 4) * 512 + c1
                        ]

                    for t_o in range(2 * HPC):
                        ps = psq_pool.tile([128, 512], f32, tag="psq")
                        for k in range(NKT):
                            nc.tensor.matmul(
                                ps[:],
                                wt(k, t_o * 128, (t_o + 1) * 128),
                                xt(k),
                                start=(k == 0),
                                stop=(k == NKT - 1),
                            )
                        ct = tb["cosq"] if t_o < HPC else tb["cosk"]
                        st = tb["sinq"] if t_o < HPC else tb["sink"]
                        qraw = rope_pool.tile([128, 512], bf16, tag="qraw")
                        nc.scalar.activation(qraw[:], ps[:], AF.Copy)
                        # partition-rotated copy (rotate_half) via ACT: single
                        # tensor-input ops may cross base partitions
                        qsw = rope_pool.tile([128, 512], bf16, tag="qsw")
                        nc.scalar.activation(qsw[0:64, :], ps[64:128, :], AF.Copy)
                        nc.scalar.activation(qsw[64:128, :], ps[0:64, :], AF.Copy)
                        tmp = rope_pool.tile([128, 512], bf16, tag="tmp")
                        half_eng.tensor_mul(
                            tmp[:], qsw[:], st[:, s0 : s0 + 512]
                        )
                        qc = rope_pool.tile([128, 512], bf16, tag="qc")
                        nc.vector.tensor_mul(qc[:], qraw[:], ct[:, s0 : s0 + 512])
                        add_eng.tensor_add(
                            qk_tiles[t_o][:, s0 : s0 + 512], qc[:], tmp[:]
                        )
                    for st_i in range(4):
                        psv = psv_pool.tile([128, 512], f32, tag="psv")
                        for k in range(NKT):
                            nc.tensor.matmul(
                                psv[:],
                                xt(k, st_i * 128, (st_i + 1) * 128),
                                wt(k, 2 * HPC * 128, 3 * HPC * 128),
                                start=(k == 0),
                                stop=(k == NKT - 1),
                            )
                        getattr(nc, cfg["qkv_evac"]).activation(
                            vsd_tiles[sc * 4 + st_i][:], psv[:], AF.Copy
                        )

            # ------------- Phase B: windowed attention -------------
            attn_tiles = []
            if "B" in cfg["phases"]:
              with ExitStack() as ph:
                pm_pool = ph.enter_context(tc.tile_pool(name="pm", bufs=cfg["pm_bufs"]))
                pr_pool = ph.enter_context(tc.tile_pool(name="pr", bufs=cfg["pr_bufs"]))
                sm_pool = ph.enter_context(tc.tile_pool(name="sm", bufs=cfg["sm_bufs"]))
                roll_pool = ph.enter_context(
                    tc.tile_pool(name="roll", bufs=cfg["roll_bufs"])
                )
                pss_pool = ph.enter_context(
                    tc.tile_pool(name="pss", bufs=cfg["pss_bufs"], space="PSUM")
                )
                pso_pool = ph.enter_context(
                    tc.tile_pool(name="pso", bufs=cfg["pso_bufs"], space="PSUM")
                )
                tr_eng = getattr(nc, cfg["tr_eng"])
                LAST_BANK = {0: 3, 1: 7, 2: 11, 3: 15}

                def setup_head(h):
                    ah = attn_pool.tile([128, S], bf16, tag="ah", name=f"ah{h}_{rep}")
                    attn_tiles.append(ah)
                    roll = roll_pool.tile(
                        [128, NSLOT * 640], bf16, tag="roll", name=f"roll{h}_{rep}"
                    )
                    roll4 = roll[:].rearrange("p (a b c) -> p a b c", b=4, c=128)
                    pieces_by_ready = {}
                    for jb in range(NST):
                        w0, w1 = jb * 128, min(jb * 128 + 640, S)
                        c = w0
                        while c < w1:
                            nxt = min(w1, (c // 512 + 1) * 512)
                            pieces_by_ready.setdefault(
                                min(jb + 4 + SHIFT, NST - 1), []
                            ).append((jb, c, nxt))
                            c = nxt
                    return dict(
                        h=h, ah=ah, roll=roll, roll4=roll4,
                        pieces=pieces_by_ready, pv_banks=[None] * 4,
                    )

                def emit_pv(stt, i):
                    h, ah, roll = stt["h"], stt["ah"], stt["roll"]
                    pv_banks = stt["pv_banks"]
                    for jb, c, nxt in stt["pieces"].get(i, ()):
                        bk = c // 512
                        if pv_banks[bk] is None:
                            pv_banks[bk] = pso_pool.tile(
                                [128, 512], f32, tag="pvo",
                                name=f"pvo_h{h}_b{bk}_{rep}",
                            )
                            nc.tensor.matmul(
                                pv_banks[bk][:], idnb, zerosb[:],
                                start=True, stop=False, skip_group_check=True,
                            )
                        last = LAST_BANK[bk] == jb
                        slot = jb % NSLOT
                        nc.tensor.matmul(
                            pv_banks[bk][:, c - bk * 512 : nxt - bk * 512],
                            vsd_tiles[jb][:, h * 128 : (h + 1) * 128],
                            roll[
                                :,
                                slot * 640 + c - jb * 128 : slot * 640 + nxt - jb * 128,
                            ],
                            start=False,
                            stop=last,
                            skip_group_check=True,
                        )
                        if last:
                            if cfg["ah_evac"] == "vector":
                                nc.vector.tensor_copy(
                                    ah[:, bk * 512 : (bk + 1) * 512], pv_banks[bk][:]
                                )
                            else:
                                nc.scalar.activation(
                                    ah[:, bk * 512 : (bk + 1) * 512],
                                    pv_banks[bk][:],
                                    AF.Copy,
                                )

                def process_block(stt, i):
                    h = stt["h"]
                    qh = qk_tiles[h]
                    kh = qk_tiles[HPC + h]
                    jlo = max(0, i * 128 - WINDOW)
                    w = i * 128 + 128 - jlo
                    mo = 640 - w
                    ps_s = pss_pool.tile([128, 1024], f32, tag="pss")
                    if cfg["edge_mask"]:
                        # mask preload only on the two 128-wide diagonal edge
                        # triangles; interior columns are fully in-window and
                        # get start=True on the score matmul itself. Order:
                        # interior-T scores, tri-T preloads, tri-F scores —
                        # every start=True re-pends its whole 2KB psum region,
                        # so all T writes must precede the F accumulates.
                        tris = []
                        if w == 640:
                            tris.append((0, 128))
                        tris.append((w - 128, w))
                        cuts = sorted(
                            {0, w, 512} | {c for t in tris for c in t}
                        )
                        cuts = [c for c in cuts if 0 <= c <= w]
                        pieces = list(zip(cuts, cuts[1:]))

                        def score_mm(c, nxt, first):
                            nc.tensor.matmul(
                                ps_s[:, c:nxt],
                                qh[:, i * 128 : (i + 1) * 128],
                                kh[:, jlo + c : jlo + nxt],
                                start=first, stop=True,
                                skip_group_check=True,
                            )

                        for c, nxt in pieces:
                            if not any(c0 <= c < c1 for c0, c1 in tris):
                                score_mm(c, nxt, True)
                        for c0, c1 in tris:
                            nc.tensor.matmul(
                                ps_s[:, c0:c1], idnb, msk[:, mo + c0 : mo + c1],
                                start=True, stop=False, skip_group_check=True,
                            )
                        for c, nxt in pieces:
                            if any(c0 <= c < c1 for c0, c1 in tris):
                                score_mm(c, nxt, False)
                    else:
                        c = 0
                        while c < w:
                            nxt = min(w, (c // 512 + 1) * 512)
                            nc.tensor.matmul(
                                ps_s[:, c:nxt], idnb, msk[:, mo + c : mo + nxt],
                                start=True, stop=False, skip_group_check=True,
                            )
                            c = nxt
                        c = 0
                        while c < w:
                            nxt = min(w, (c // 512 + 1) * 512)
                            nc.tensor.matmul(
                                ps_s[:, c:nxt],
                                qh[:, i * 128 : (i + 1) * 128],
                                kh[:, jlo + c : jlo + nxt],
                                start=False, stop=True, skip_group_check=True,
                            )
                            c = nxt
                    pm = pm_pool.tile([128, 640], bf16, tag="pm")
                    sums = sm_pool.tile([128, 1], f32, tag="sums")
                    nc.scalar.activation(
                        pm[:, :w], ps_s[:, :w], AF.Exp, accum_out=sums[:]
                    )
                    rc = sm_pool.tile([128, 1], f32, tag="rc")
                    nc.vector.reciprocal(rc[:], sums[:])
                    pr = pr_pool.tile([128, 640], bf16, tag="pr")
                    if cfg["norm_eng"] == "vector":
                        nc.vector.tensor_scalar_mul(pr[:, :w], pm[:, :w], rc[:])
                    else:
                        nc.scalar.activation(pr[:, :w], pm[:, :w], AF.Copy, scale=rc[:])
                    # transposes into roll slots on the DMA xbar; the dst for
                    # slice z is slot (j0+z)%8 at column (i-j0-z)*128, which is
                    # a uniform +512-element progression between wraps -> at
                    # most 2 dma_start_transpose instrs per block.
                    j0 = jlo // 128
                    nblk = w // 128
                    roll4 = stt["roll4"]
                    z = 0
                    while z < nblk:
                        s0_ = (j0 + z) % NSLOT
                        zlen = min(nblk - z, NSLOT - s0_)
                        base = s0_ * 640 + (i - j0 - z) * 128
                        a0, b0 = base // 512, (base % 512) // 128
                        tr_eng.dma_start_transpose(
                            roll4[:, a0 : a0 + zlen, b0 : b0 + 1, :],
                            pr[:, z * 128 : (z + zlen) * 128],
                        )
                        z += zlen
                    emit_pv(stt, i)

                for hg in range(HPC // HG):
                    states = [setup_head(hg * HG + u) for u in range(HG)]
                    for i in range(NST):
                        for stt in states:
                            process_block(stt, i)

            # ------------- Phase C: output projection -------------
            if "C" in cfg["phases"]:
              with ExitStack() as ph:
                ob_pool = ph.enter_context(tc.tile_pool(name="ob", bufs=cfg["ob_bufs"]))
                psc_pool = ph.enter_context(
                    tc.tile_pool(name="psc", bufs=cfg["psc_bufs"], space="PSUM")
                )
                for st_i in range(NST):
                    ob = ob_pool.tile([128, HIDDEN], bf16, tag="ob")
                    for mc in range(HIDDEN // 512):
                        ps = psc_pool.tile([128, 512], f32, tag="psc")
                        for hh in range(HPC):
                            nc.tensor.matmul(
                                ps[:],
                                attn_tiles[hh][:, st_i * 128 : (st_i + 1) * 128],
                                wo_all[
                                    :, hh * HIDDEN + mc * 512 : hh * HIDDEN + (mc + 1) * 512
                                ],
                                start=(hh == 0),
                                stop=(hh == HPC - 1),
                            )
                        if cfg["c_evac_alt"] and mc % 2 == 1:
                            nc.scalar.activation(
                                ob[:, mc * 512 : (mc + 1) * 512], ps[:], AF.Copy
                            )
                        else:
                            nc.vector.tensor_copy(
                                ob[:, mc * 512 : (mc + 1) * 512], ps[:]
                            )
                    getattr(nc, cfg["out_eng"]).dma_start(
                        out_d[st_i * 128 : (st_i + 1) * 128, :], ob[:]
                    )

    nc.compile()
    return nc


def _get_module(repeat=1, cfg=None):
    key = ("nc", repeat, tuple(sorted((cfg or {}).items())))
    if key not in _CACHE:
        _CACHE[key] = _build_module(repeat, cfg)
    return _CACHE[key]


def make_in_maps(hidden_states, cos, sin, w_qkv, w_o):
    import ml_dtypes

    bf = ml_dtypes.bfloat16
    hidden_states = np.asarray(hidden_states, dtype=np.float32)
    cos = np.asarray(cos, dtype=np.float32)
    sin = np.asarray(sin, dtype=np.float32)
    w_qkv = np.asarray(w_qkv, dtype=np.float32)
    w_o = np.asarray(w_o, dtype=np.float32)

    cosT = np.ascontiguousarray(cos.T)  # [DH, S]
    sinT = np.ascontiguousarray(sin.T)
    sinS = sinT.copy()
    sinS[: DH // 2] *= -1.0  # fold rotate_half sign
    tbls = np.concatenate(
        [cosT * SCALE, sinS * SCALE, cosT, sinS], axis=1
    ).astype(bf)

    qi = np.arange(128)[:, None]
    jj = np.arange(640)[None, :]
    mask = np.where((jj > qi) & (jj <= qi + WINDOW), 0.0, NEG).astype(np.float32)
    cmask = np.concatenate([mask, np.eye(128, dtype=np.float32)], axis=1).astype(bf)

    xTs = [np.ascontiguousarray(hidden_states[b].T).astype(bf) for b in range(B)]

    in_maps = []
    for c in range(N_CORES):
        b, hg = divmod(c, N_CORES // B)
        r0 = hg * HPC * DH
        wq = w_qkv[r0 : r0 + HPC * DH]
        wk = w_qkv[N_HEADS * DH + r0 : N_HEADS * DH + r0 + HPC * DH]
        wv = w_qkv[2 * N_HEADS * DH + r0 : 2 * N_HEADS * DH + r0 + HPC * DH]
        wTc = np.ascontiguousarray(np.concatenate([wq, wk, wv], axis=0).T).astype(bf)
        # wo: per head-slice [128, HIDDEN], concatenated along columns
        woT = w_o[:, r0 : r0 + HPC * DH].T  # [512, HIDDEN]
        wo_cat = np.concatenate(
            [woT[hh * 128 : (hh + 1) * 128] for hh in range(HPC)], axis=1
        ).astype(bf)
        in_maps.append(
            {
                "xT": xTs[b],
                "wT": wTc,
                "wo": np.ascontiguousarray(wo_cat),
                "tbls": tbls,
                "cmask": cmask,
            }
        )
    return in_maps


def gather(results):
    out = np.zeros((B, S, HIDDEN), dtype=np.float32)
    for c in range(N_CORES):
        b = c // (N_CORES // B)
        out[b] += results[c]["out"].astype(np.float32)
    return out


def kernel(hidden_states, cos, sin, w_qkv, w_o):
    from concourse.bass_utils import run_bass_kernel_spmd

    nc = _get_module()
    in_maps = make_in_maps(hidden_states, cos, sin, w_qkv, w_o)
    res = run_bass_kernel_spmd(nc, in_maps, list(range(N_CORES)))
    return gather(res.results)



# revision 8
# speedup vs baseline: 1.3588x; 1.3588x over previous
"""Sliding-window attention v3: fused B+C phases, per-q-tile PV, packed pso,
DMA prologue reorder. See kernel.py (v2) for the base structure.

v3 vs v2:
- PV emitted per q-tile (all 5 jb pieces ready after block t's transpose),
  packed 4 heads x 128q into one PSUM bank -> pso 2 banks instead of 4+
- o_proj fused into the attention loop (lagged 2 segments) so PE fills
  phase-B latency gaps with dense o_proj matmuls
- edge_mask default on (mask preload only the 2 diagonal triangles)
- softmax norm on DVE (ACT was the phase-B throughput limit)
- prologue: wt DMA split into 4 chunks, const DMAs (tbl/cm/wo) issued
  after the first xt/wt chunks so PE starts ~6us in instead of ~33us
- the v2 roll ring is gone: per-q-tile PV only ever reads block t's own
  P^T, so each block transposes into a single contiguous stage tile
  (one dma_start_transpose per block instead of ~2)
"""
import sys

sys.path.insert(0, "/opt/trn_rl_repo")

import numpy as np

B = 2
S = 2048
HIDDEN = 2048
N_HEADS = 16
DH = 128
WINDOW = 512
HPC = 4  # heads per core
N_CORES = 8
QKV_O = 3 * HPC * DH  # 1536
SCALE = 1.0 / np.sqrt(DH)
NEG = -1.0e30

_CACHE = {}

CFG = {
    "psq_bufs": 6, "psv_bufs": 2,
    "xt_bufs": 6, "rope_bufs": 3,
    "pm_bufs": 3, "pr_bufs": 3, "sm_bufs": 8,
    "pss_bufs": 2, "pso_bufs": 2, "psc_bufs": 2, "ob_bufs": 2,
    "stg_bufs": 2,
    "qkv_evac": "scalar",
    "rope_half_eng": "gpsimd", "rope_add_eng": "vector",
    "norm_eng": "vector",
    "pso_evac": "vector",
    "c_evac_alt": True,
    "edge_mask": True,
    "mask_eng": "pe",   # pe: idn-matmul preload | gpsimd/vector: psum add post-scores
    "pv_zero": "pe",    # pe: idn@zeros matmul | gpsimd/vector: engine memset
    "lag": 1,    # segments between scores(t) and PV(t)
    "olag": 2,   # segments between scores(t) and o_proj(t)
    "order": "scores_first",  # per-segment emission order: spread | scores_first
    # start=True per sub-region is NOT safe: start_tensor_calc pends the whole
    # 2KB psum bank, so a later head's start would corrupt earlier heads'
    # accumulation. Keep the single 512-wide zero-preload matmul per q-tile.
    "pv_start": False,
    "tr_eng": "sync", "xt_eng": "sync", "out_eng": "sync",
    "wt_chunks": 4,
    "phases": "ABC",
}


def _build_module(repeat=1, cfg=None):
    cfg = {**CFG, **(cfg or {})}
    import concourse.tile as tile
    from concourse import bacc, mybir
    from contextlib import ExitStack

    f32 = mybir.dt.float32
    bf16 = mybir.dt.bfloat16
    AF = mybir.ActivationFunctionType

    nc = bacc.Bacc("TRN2", target_bir_lowering=False, debug=False)

    xT = nc.declare_dram_parameter("xT", [HIDDEN, S], bf16, isOutput=False)
    wT = nc.declare_dram_parameter("wT", [HIDDEN, QKV_O], bf16, isOutput=False)
    wo_d = nc.declare_dram_parameter("wo", [128, HPC * HIDDEN], bf16, isOutput=False)
    tbl_d = nc.declare_dram_parameter("tbls", [DH, 4 * S], bf16, isOutput=False)
    cm_d = nc.declare_dram_parameter("cmask", [128, 768], bf16, isOutput=False)
    out_d = nc.declare_dram_parameter("out", [S, HIDDEN], bf16, isOutput=True)

    NKT = HIDDEN // 128  # 16 contraction chunks
    NSC = S // 512  # 4 sequence chunks
    NST = S // 128  # 16 sequence tiles
    LAG = cfg["lag"]
    OLAG = cfg["olag"]
    WTC = cfg["wt_chunks"]

    with tile.TileContext(nc) as tc, ExitStack() as top:
        cpool = top.enter_context(tc.tile_pool(name="consts", bufs=1))
        cm = cpool.tile([128, 768], bf16, tag="cmask")
        msk = cm[:, 0:640]
        idnb = cm[:, 640:768]
        wo_all = cpool.tile([128, HPC * HIDDEN], bf16, tag="wo")
        tbl = cpool.tile([DH, 4 * S], bf16, tag="tbls")
        wt_all = cpool.tile([128, NKT * QKV_O], bf16, tag="wt")
        zerosb = None
        if not cfg["pv_start"]:
            zerosb = cpool.tile([128, 512], bf16, tag="zerosb")
            nc.vector.memset(zerosb[:], 0)
        tb = {
            "cosq": tbl[:, 0 * S : 1 * S],
            "sinq": tbl[:, 1 * S : 2 * S],
            "cosk": tbl[:, 2 * S : 3 * S],
            "sink": tbl[:, 3 * S : 4 * S],
        }

        def wt(k, c0, c1):
            return wt_all[:, k * QKV_O + c0 : k * QKV_O + c1]

        # persistent q/k (post-rope), v, and attention-out tiles
        qk_pool = top.enter_context(tc.tile_pool(name="qk", bufs=1))
        vsd_pool = top.enter_context(tc.tile_pool(name="vsd", bufs=NST))
        ah_pool = top.enter_context(tc.tile_pool(name="ah", bufs=1))

        wv = wT[:].rearrange("(k p) c -> p k c", p=128)
        xv = xT[:].rearrange("(k p) s -> p k s", p=128)

        for rep in range(repeat):
            qk_tiles = [
                qk_pool.tile([128, S], bf16, tag=f"qk{t_o}", name=f"qk{t_o}_{rep}")
                for t_o in range(2 * HPC)
            ]
            vsd_tiles = [
                vsd_pool.tile([128, HPC * DH], bf16, tag="vsd", name=f"vsd{st}_{rep}")
                for st in range(NST)
            ]
            # attention out, [d, q] packed as (t, h, 128q) along columns
            ah_all = ah_pool.tile([128, NST * 512], bf16, tag="ah", name=f"ah_{rep}")

            # ------------- Phase A: QKV projection + RoPE -------------
            if "A" in cfg["phases"]:
              with ExitStack() as ph:
                xt_pool = ph.enter_context(
                    tc.tile_pool(name="xt", bufs=cfg["xt_bufs"])
                )
                rope_pool = ph.enter_context(
                    tc.tile_pool(name="rope", bufs=cfg["rope_bufs"])
                )
                psq_pool = ph.enter_context(
                    tc.tile_pool(name="psq", bufs=cfg["psq_bufs"], space="PSUM")
                )
                psv_pool = ph.enter_context(
                    tc.tile_pool(name="psv", bufs=cfg["psv_bufs"], space="PSUM")
                )

                half_eng = getattr(nc, cfg["rope_half_eng"])
                add_eng = getattr(nc, cfg["rope_add_eng"])

                for sc in range(NSC):
                    s0 = sc * 512
                    xt_q = []
                    for qf in range(4):
                        t = xt_pool.tile([128, 4 * 512], bf16, tag="xt")
                        getattr(nc, cfg["xt_eng"]).dma_start(
                            t[:], xv[:, qf * 4 : qf * 4 + 4, s0 : s0 + 512]
                        )
                        xt_q.append(t)
                        if rep == 0 and sc == 0:
                            # interleave weight chunks with the first x tiles
                            # so PE can start after ~2MB instead of ~11MB
                            kpc = NKT // WTC
                            if qf < WTC:
                                nc.sync.dma_start(
                                    wt_all[
                                        :,
                                        qf * kpc * QKV_O : (qf + 1) * kpc * QKV_O,
                                    ],
                                    wv[:, qf * kpc : (qf + 1) * kpc, :],
                                )
                    if rep == 0 and sc == 0:
                        nc.sync.dma_start(tbl[:], tbl_d[:])
                        nc.sync.dma_start(cm[:], cm_d[:])
                        nc.sync.dma_start(wo_all[:], wo_d[:])

                    def xt(k, c0=0, c1=512):
                        return xt_q[k // 4][
                            :, (k % 4) * 512 + c0 : (k % 4) * 512 + c1
                        ]

                    for t_o in range(2 * HPC):
                        ps = psq_pool.tile([128, 512], f32, tag="psq")
                        for k in range(NKT):
                            nc.tensor.matmul(
                                ps[:],
                                wt(k, t_o * 128, (t_o + 1) * 128),
                                xt(k),
                                start=(k == 0),
                                stop=(k == NKT - 1),
                            )
                        ct = tb["cosq"] if t_o < HPC else tb["cosk"]
                        st = tb["sinq"] if t_o < HPC else tb["sink"]
                        qraw = rope_pool.tile([128, 512], bf16, tag="qraw")
                        nc.scalar.activation(qraw[:], ps[:], AF.Copy)
                        # partition-rotated copy (rotate_half) via ACT: single
                        # tensor-input ops may cross base partitions
                        qsw = rope_pool.tile([128, 512], bf16, tag="qsw")
                        nc.scalar.activation(qsw[0:64, :], ps[64:128, :], AF.Copy)
                        nc.scalar.activation(qsw[64:128, :], ps[0:64, :], AF.Copy)
                        tmp = rope_pool.tile([128, 512], bf16, tag="tmp")
                        half_eng.tensor_mul(
                            tmp[:], qsw[:], st[:, s0 : s0 + 512]
                        )
                        qc = rope_pool.tile([128, 512], bf16, tag="qc")
                        nc.vector.tensor_mul(qc[:], qraw[:], ct[:, s0 : s0 + 512])
                        add_eng.tensor_add(
                            qk_tiles[t_o][:, s0 : s0 + 512], qc[:], tmp[:]
                        )
                    for st_i in range(4):
                        psv = psv_pool.tile([128, 512], f32, tag="psv")
                        for k in range(NKT):
                            nc.tensor.matmul(
                                psv[:],
                                xt(k, st_i * 128, (st_i + 1) * 128),
                                wt(k, 2 * HPC * 128, 3 * HPC * 128),
                                start=(k == 0),
                                stop=(k == NKT - 1),
                            )
                        getattr(nc, cfg["qkv_evac"]).activation(
                            vsd_tiles[sc * 4 + st_i][:], psv[:], AF.Copy
                        )

            # ------------- Phase B+C: windowed attention + o_proj -------------
            if "B" in cfg["phases"]:
              with ExitStack() as ph:
                pm_pool = ph.enter_context(tc.tile_pool(name="pm", bufs=cfg["pm_bufs"]))
                pr_pool = ph.enter_context(tc.tile_pool(name="pr", bufs=cfg["pr_bufs"]))
                sm_pool = ph.enter_context(tc.tile_pool(name="sm", bufs=cfg["sm_bufs"]))
                stg_pool = ph.enter_context(
                    tc.tile_pool(name="stg", bufs=cfg["stg_bufs"])
                )
                pss_pool = ph.enter_context(
                    tc.tile_pool(name="pss", bufs=cfg["pss_bufs"], space="PSUM")
                )
                pso_pool = ph.enter_context(
                    tc.tile_pool(name="pso", bufs=cfg["pso_bufs"], space="PSUM")
                )
                psc_pool = ph.enter_context(
                    tc.tile_pool(name="psc", bufs=cfg["psc_bufs"], space="PSUM")
                )
                ob_pool = ph.enter_context(tc.tile_pool(name="ob", bufs=cfg["ob_bufs"]))
                tr_eng = getattr(nc, cfg["tr_eng"])
                # stages[h]: P^T of the current block, [j-within-tile, (jtile q)]
                stages = [[None] * NST for _ in range(HPC)]

                def process_block(h, i):
                    qh = qk_tiles[h]
                    kh = qk_tiles[HPC + h]
                    jlo = max(0, i * 128 - WINDOW)
                    w = i * 128 + 128 - jlo
                    mo = 640 - w
                    ps_s = pss_pool.tile([128, 640], f32, tag="pss")
                    if cfg["mask_eng"] != "pe":
                        # scores everywhere with start=True (split at the 512
                        # psum-bank boundary), then the idle Pool/DVE engine
                        # adds the NEG mask onto the two diagonal triangles
                        tris = [(w - 128, w)]
                        if w == 640:
                            tris.insert(0, (0, 128))
                        c = 0
                        while c < w:
                            nxt = min(w, (c // 512 + 1) * 512)
                            nc.tensor.matmul(
                                ps_s[:, c:nxt],
                                qh[:, i * 128 : (i + 1) * 128],
                                kh[:, jlo + c : jlo + nxt],
                                start=True, stop=True, skip_group_check=True,
                            )
                            c = nxt
                        meng = getattr(nc, cfg["mask_eng"])
                        for c0, c1 in tris:
                            meng.tensor_add(
                                ps_s[:, c0:c1],
                                ps_s[:, c0:c1],
                                msk[:, mo + c0 : mo + c1],
                            )
                    elif cfg["edge_mask"]:
                        # mask preload only on the two 128-wide diagonal edge
                        # triangles; interior columns get start=True on the
                        # score matmul itself. All start=True writes into the
                        # psum region must precede the start=False ones.
                        tris = []
                        if w == 640:
                            tris.append((0, 128))
                        tris.append((w - 128, w))
                        cuts = sorted(
                            {0, w, 512} | {c for t in tris for c in t}
                        )
                        cuts = [c for c in cuts if 0 <= c <= w]
                        pieces = list(zip(cuts, cuts[1:]))

                        def score_mm(c, nxt, first):
                            nc.tensor.matmul(
                                ps_s[:, c:nxt],
                                qh[:, i * 128 : (i + 1) * 128],
                                kh[:, jlo + c : jlo + nxt],
                                start=first, stop=True,
                                skip_group_check=True,
                            )

                        for c, nxt in pieces:
                            if not any(c0 <= c < c1 for c0, c1 in tris):
                                score_mm(c, nxt, True)
                        for c0, c1 in tris:
                            nc.tensor.matmul(
                                ps_s[:, c0:c1], idnb, msk[:, mo + c0 : mo + c1],
                                start=True, stop=False, skip_group_check=True,
                            )
                        for c, nxt in pieces:
                            if any(c0 <= c < c1 for c0, c1 in tris):
                                score_mm(c, nxt, False)
                    else:
                        c = 0
                        while c < w:
                            nxt = min(w, (c // 512 + 1) * 512)
                            nc.tensor.matmul(
                                ps_s[:, c:nxt], idnb, msk[:, mo + c : mo + nxt],
                                start=True, stop=False, skip_group_check=True,
                            )
                            c = nxt
                        c = 0
                        while c < w:
                            nxt = min(w, (c // 512 + 1) * 512)
                            nc.tensor.matmul(
                                ps_s[:, c:nxt],
                                qh[:, i * 128 : (i + 1) * 128],
                                kh[:, jlo + c : jlo + nxt],
                                start=False, stop=True, skip_group_check=True,
                            )
                            c = nxt
                    pm = pm_pool.tile([128, 640], bf16, tag="pm")
                    sums = sm_pool.tile([128, 1], f32, tag="sums")
                    nc.scalar.activation(
                        pm[:, :w], ps_s[:, :w], AF.Exp, accum_out=sums[:]
                    )
                    rc = sm_pool.tile([128, 1], f32, tag="rc")
                    nc.vector.reciprocal(rc[:], sums[:])
                    pr = pr_pool.tile([128, 640], bf16, tag="pr")
                    if cfg["norm_eng"] == "vector":
                        nc.vector.tensor_scalar_mul(pr[:, :w], pm[:, :w], rc[:])
                    else:
                        nc.scalar.activation(pr[:, :w], pm[:, :w], AF.Copy, scale=rc[:])
                    # P^T for this block on the DMA xbar: one instr into a
                    # contiguous stage tile [j-within-tile, (jtile, q)]
                    nblk = w // 128
                    stg = stg_pool.tile(
                        [128, 640], bf16, tag=f"stg{h}", name=f"stg{h}_{i}_{rep}"
                    )
                    stages[h][i] = stg
                    stg3 = stg[:].rearrange("p (a c) -> p a c", c=128)
                    tr_eng.dma_start_transpose(
                        stg3[:, 0:nblk, :], pr[:, 0 : nblk * 128]
                    )

                def emit_pv(t):
                    # all 5 jb pieces for q-tile t are written by block t's
                    # transposes; 4 heads packed into one PSUM bank
                    psot = pso_pool.tile(
                        [128, 512], f32, tag="psot", name=f"psot{t}_{rep}"
                    )
                    jlo_t = max(0, t - 4)
                    if cfg["pv_zero"] != "pe":
                        getattr(nc, cfg["pv_zero"]).memset(psot[:], 0)
                    elif not cfg["pv_start"]:
                        nc.tensor.matmul(
                            psot[:], idnb, zerosb[:],
                            start=True, stop=False, skip_group_check=True,
                        )

                    def piece(h, jb, first):
                        z = jb - jlo_t
                        nc.tensor.matmul(
                            psot[:, h * 128 : (h + 1) * 128],
                            vsd_tiles[jb][:, h * 128 : (h + 1) * 128],
                            stages[h][t][:, z * 128 : (z + 1) * 128],
                            start=first and cfg["pv_start"],
                            stop=(jb == t),
                            skip_group_check=True,
                        )

                    # all start=True writes (first jb per head) first, then
                    # the accumulating ones
                    if cfg["pv_start"]:
                        for h in range(HPC):
                            piece(h, jlo_t, True)
                        rest = range(jlo_t + 1, t + 1)
                    else:
                        rest = range(jlo_t, t + 1)
                    for jb in rest:
                        for h in range(HPC):
                            piece(h, jb, False)
                    if cfg["pso_evac"] == "vector":
                        nc.vector.tensor_copy(
                            ah_all[:, t * 512 : (t + 1) * 512], psot[:]
                        )
                    else:
                        nc.scalar.activation(
                            ah_all[:, t * 512 : (t + 1) * 512], psot[:], AF.Copy
                        )

                def emit_oproj(t, ob, mcs):
                    for mc in mcs:
                        ps = psc_pool.tile([128, 512], f32, tag="psc")
                        for hh in range(HPC):
                            nc.tensor.matmul(
                                ps[:],
                                ah_all[
                                    :, (t * HPC + hh) * 128 : (t * HPC + hh + 1) * 128
                                ],
                                wo_all[
                                    :, hh * HIDDEN + mc * 512 : hh * HIDDEN + (mc + 1) * 512
                                ],
                                start=(hh == 0),
                                stop=(hh == HPC - 1),
                            )
                        if cfg["c_evac_alt"] and mc % 2 == 1:
                            nc.scalar.activation(
                                ob[:, mc * 512 : (mc + 1) * 512], ps[:], AF.Copy
                            )
                        else:
                            nc.vector.tensor_copy(
                                ob[:, mc * 512 : (mc + 1) * 512], ps[:]
                            )
                    if mcs[-1] == HIDDEN // 512 - 1:
                        getattr(nc, cfg["out_eng"]).dma_start(
                            out_d[t * 128 : (t + 1) * 128, :], ob[:]
                        )

                # Emission order interleaves scores / PV / o_proj so the
                # in-order PE queue never stalls at its head: while ACT runs
                # exp(h0,h1), PE does PV(t-LAG); while exp(h2,h3) run, PE does
                # o_proj(t-OLAG).
                for seg in range(NST + OLAG):
                    t = seg - LAG
                    to = seg - OLAG
                    ob = None
                    if 0 <= to < NST:
                        ob = ob_pool.tile(
                            [128, HIDDEN], bf16, tag="ob", name=f"ob{to}_{rep}"
                        )
                    if cfg["order"] == "spread":
                        if seg < NST:
                            process_block(0, seg)
                            process_block(1, seg)
                        if 0 <= t < NST:
                            emit_pv(t)
                        if seg < NST:
                            process_block(2, seg)
                        if ob is not None:
                            emit_oproj(to, ob, [0, 1])
                        if seg < NST:
                            process_block(3, seg)
                        if ob is not None:
                            emit_oproj(to, ob, [2, 3])
                    else:  # scores_first
                        if seg < NST:
                            process_block(0, seg)
                            process_block(1, seg)
                        if 0 <= t < NST:
                            emit_pv(t)
                        if seg < NST:
                            process_block(2, seg)
                            process_block(3, seg)
                        if ob is not None:
                            emit_oproj(to, ob, [0, 1, 2, 3])

    nc.compile()
    return nc


def _get_module(repeat=1, cfg=None):
    key = ("nc", repeat, tuple(sorted((cfg or {}).items())))
    if key not in _CACHE:
        _CACHE[key] = _build_module(repeat, cfg)
    return _CACHE[key]


def make_in_maps(hidden_states, cos, sin, w_qkv, w_o):
    import ml_dtypes

    bf = ml_dtypes.bfloat16
    hidden_states = np.asarray(hidden_states, dtype=np.float32)
    cos = np.asarray(cos, dtype=np.float32)
    sin = np.asarray(sin, dtype=np.float32)
    w_qkv = np.asarray(w_qkv, dtype=np.float32)
    w_o = np.asarray(w_o, dtype=np.float32)

    cosT = np.ascontiguousarray(cos.T)  # [DH, S]
    sinT = np.ascontiguousarray(sin.T)
    sinS = sinT.copy()
    sinS[: DH // 2] *= -1.0  # fold rotate_half sign
    tbls = np.concatenate(
        [cosT * SCALE, sinS * SCALE, cosT, sinS], axis=1
    ).astype(bf)

    qi = np.arange(128)[:, None]
    jj = np.arange(640)[None, :]
    mask = np.where((jj > qi) & (jj <= qi + WINDOW), 0.0, NEG).astype(np.float32)
    cmask = np.concatenate([mask, np.eye(128, dtype=np.float32)], axis=1).astype(bf)

    xTs = [np.ascontiguousarray(hidden_states[b].T).astype(bf) for b in range(B)]

    in_maps = []
    for c in range(N_CORES):
        b, hg = divmod(c, N_CORES // B)
        r0 = hg * HPC * DH
        wq = w_qkv[r0 : r0 + HPC * DH]
        wk = w_qkv[N_HEADS * DH + r0 : N_HEADS * DH + r0 + HPC * DH]
        wv = w_qkv[2 * N_HEADS * DH + r0 : 2 * N_HEADS * DH + r0 + HPC * DH]
        wTc = np.ascontiguousarray(np.concatenate([wq, wk, wv], axis=0).T).astype(bf)
        # wo: per head-slice [128, HIDDEN], concatenated along columns
        woT = w_o[:, r0 : r0 + HPC * DH].T  # [512, HIDDEN]
        wo_cat = np.concatenate(
            [woT[hh * 128 : (hh + 1) * 128] for hh in range(HPC)], axis=1
        ).astype(bf)
        in_maps.append(
            {
                "xT": xTs[b],
                "wT": wTc,
                "wo": np.ascontiguousarray(wo_cat),
                "tbls": tbls,
                "cmask": cmask,
            }
        )
    return in_maps


def gather(results):
    out = np.zeros((B, S, HIDDEN), dtype=np.float32)
    for c in range(N_CORES):
        b = c // (N_CORES // B)
        out[b] += results[c]["out"].astype(np.float32)
    return out


def kernel(hidden_states, cos, sin, w_qkv, w_o):
    from concourse.bass_utils import run_bass_kernel_spmd

    nc = _get_module()
    in_maps = make_in_maps(hidden_states, cos, sin, w_qkv, w_o)
    res = run_bass_kernel_spmd(nc, in_maps, list(range(N_CORES)))
    return gather(res.results)


# revision 9
# speedup vs baseline: 1.3841x; 1.0186x over previous
"""Sliding-window attention v4: full A<->B+C interleave.

v4 vs v3: the attention+o_proj segments for sequence chunk sc-1 are emitted
between the QKV projection groups of chunk sc, so every cross-engine latency
chain (scores -> exp -> norm -> transpose -> PV -> evac -> o_proj) hides under
the dense QKV matmul stream instead of stalling the in-order PE queue.

PSUM budget (8 banks): psq ring shared by q/k/v projection groups (2), score
psum [128,640] x2 (4), one ring shared by PV-out and o_proj accumulators (2).
"""
import sys

sys.path.insert(0, "/opt/trn_rl_repo")

import numpy as np

B = 2
S = 2048
HIDDEN = 2048
N_HEADS = 16
DH = 128
WINDOW = 512
HPC = 4  # heads per core
N_CORES = 8
QKV_O = 3 * HPC * DH  # 1536
SCALE = 1.0 / np.sqrt(DH)
NEG = -1.0e30

_CACHE = {}

CFG = {
    "psq_bufs": 2, "pss_bufs": 2, "px_bufs": 2,
    "xt_bufs": 5, "rope_bufs": 2,
    "pm_bufs": 3, "pr_bufs": 3, "sm_bufs": 8,
    "ob_bufs": 2, "stg_bufs": 2,
    "qkv_evac": "scalar",
    "rope_half_eng": "gpsimd", "rope_add_eng": "vector",
    "norm_eng": "vector",
    "pso_evac": "vector",
    "c_evac_alt": True,
    "edge_mask": True,
    "lag": 1,    # segments between scores(t) and PV(t)
    "olag": 2,   # segments between scores(t) and o_proj(t)
    "tr_eng": "sync", "xt_eng": "sync", "out_eng": "sync",
    "wt_chunks": 4,
    "gap": 3,    # A-groups between consecutive segments in the interleave
}


def _build_module(repeat=1, cfg=None):
    cfg = {**CFG, **(cfg or {})}
    import concourse.tile as tile
    from concourse import bacc, mybir
    from contextlib import ExitStack

    f32 = mybir.dt.float32
    bf16 = mybir.dt.bfloat16
    AF = mybir.ActivationFunctionType

    nc = bacc.Bacc("TRN2", target_bir_lowering=False, debug=False)

    xT = nc.declare_dram_parameter("xT", [HIDDEN, S], bf16, isOutput=False)
    wT = nc.declare_dram_parameter("wT", [HIDDEN, QKV_O], bf16, isOutput=False)
    wo_d = nc.declare_dram_parameter("wo", [128, HPC * HIDDEN], bf16, isOutput=False)
    tbl_d = nc.declare_dram_parameter("tbls", [DH, 4 * S], bf16, isOutput=False)
    cm_d = nc.declare_dram_parameter("cmask", [128, 768], bf16, isOutput=False)
    out_d = nc.declare_dram_parameter("out", [S, HIDDEN], bf16, isOutput=True)

    NKT = HIDDEN // 128  # 16 contraction chunks
    NSC = S // 512  # 4 sequence chunks
    NST = S // 128  # 16 sequence tiles
    LAG = cfg["lag"]
    OLAG = cfg["olag"]
    WTC = cfg["wt_chunks"]

    with tile.TileContext(nc) as tc, ExitStack() as top:
        cpool = top.enter_context(tc.tile_pool(name="consts", bufs=1))
        cm = cpool.tile([128, 768], bf16, tag="cmask")
        msk = cm[:, 0:640]
        idnb = cm[:, 640:768]
        wo_all = cpool.tile([128, HPC * HIDDEN], bf16, tag="wo")
        tbl = cpool.tile([DH, 4 * S], bf16, tag="tbls")
        wt_all = cpool.tile([128, NKT * QKV_O], bf16, tag="wt")
        zerosb = cpool.tile([128, 512], bf16, tag="zerosb")
        nc.vector.memset(zerosb[:], 0)
        tb = {
            "cosq": tbl[:, 0 * S : 1 * S],
            "sinq": tbl[:, 1 * S : 2 * S],
            "cosk": tbl[:, 2 * S : 3 * S],
            "sink": tbl[:, 3 * S : 4 * S],
        }

        def wt(k, c0, c1):
            return wt_all[:, k * QKV_O + c0 : k * QKV_O + c1]

        qk_pool = top.enter_context(tc.tile_pool(name="qk", bufs=1))
        vsd_pool = top.enter_context(tc.tile_pool(name="vsd", bufs=NST))
        ah_pool = top.enter_context(tc.tile_pool(name="ah", bufs=1))

        wv = wT[:].rearrange("(k p) c -> p k c", p=128)
        xv = xT[:].rearrange("(k p) s -> p k s", p=128)

        for rep in range(repeat):
            qk_tiles = [
                qk_pool.tile([128, S], bf16, tag=f"qk{t_o}", name=f"qk{t_o}_{rep}")
                for t_o in range(2 * HPC)
            ]
            vsd_tiles = [
                vsd_pool.tile([128, HPC * DH], bf16, tag="vsd", name=f"vsd{st}_{rep}")
                for st in range(NST)
            ]
            ah_all = ah_pool.tile([128, NST * 512], bf16, tag="ah", name=f"ah_{rep}")

            with ExitStack() as ph:
                xt_pool = ph.enter_context(
                    tc.tile_pool(name="xt", bufs=cfg["xt_bufs"])
                )
                rope_pool = ph.enter_context(
                    tc.tile_pool(name="rope", bufs=cfg["rope_bufs"])
                )
                pm_pool = ph.enter_context(tc.tile_pool(name="pm", bufs=cfg["pm_bufs"]))
                pr_pool = ph.enter_context(tc.tile_pool(name="pr", bufs=cfg["pr_bufs"]))
                sm_pool = ph.enter_context(tc.tile_pool(name="sm", bufs=cfg["sm_bufs"]))
                stg_pool = ph.enter_context(
                    tc.tile_pool(name="stg", bufs=cfg["stg_bufs"])
                )
                ob_pool = ph.enter_context(tc.tile_pool(name="ob", bufs=cfg["ob_bufs"]))
                psq_pool = ph.enter_context(
                    tc.tile_pool(name="psq", bufs=cfg["psq_bufs"], space="PSUM")
                )
                pss_pool = ph.enter_context(
                    tc.tile_pool(name="pss", bufs=cfg["pss_bufs"], space="PSUM")
                )
                px_pool = ph.enter_context(
                    tc.tile_pool(name="px", bufs=cfg["px_bufs"], space="PSUM")
                )

                half_eng = getattr(nc, cfg["rope_half_eng"])
                add_eng = getattr(nc, cfg["rope_add_eng"])
                tr_eng = getattr(nc, cfg["tr_eng"])
                stages = [[None] * NST for _ in range(HPC)]
                xt_cur = []  # current chunk's x tiles

                def emit_xt(sc):
                    s0 = sc * 512
                    del xt_cur[:]
                    for qf in range(4):
                        t = xt_pool.tile([128, 4 * 512], bf16, tag="xt")
                        getattr(nc, cfg["xt_eng"]).dma_start(
                            t[:], xv[:, qf * 4 : qf * 4 + 4, s0 : s0 + 512]
                        )
                        xt_cur.append(t)
                        if rep == 0 and sc == 0:
                            kpc = NKT // WTC
                            if qf < WTC:
                                nc.sync.dma_start(
                                    wt_all[
                                        :, qf * kpc * QKV_O : (qf + 1) * kpc * QKV_O
                                    ],
                                    wv[:, qf * kpc : (qf + 1) * kpc, :],
                                )
                    if rep == 0 and sc == 0:
                        nc.sync.dma_start(tbl[:], tbl_d[:])
                        nc.sync.dma_start(cm[:], cm_d[:])
                        nc.sync.dma_start(wo_all[:], wo_d[:])

                def xt(k, c0=0, c1=512):
                    return xt_cur[k // 4][:, (k % 4) * 512 + c0 : (k % 4) * 512 + c1]

                def emit_qk_group(sc, t_o):
                    s0 = sc * 512
                    ps = psq_pool.tile([128, 512], f32, tag="psq")
                    for k in range(NKT):
                        nc.tensor.matmul(
                            ps[:],
                            wt(k, t_o * 128, (t_o + 1) * 128),
                            xt(k),
                            start=(k == 0),
                            stop=(k == NKT - 1),
                        )
                    ct = tb["cosq"] if t_o < HPC else tb["cosk"]
                    st = tb["sinq"] if t_o < HPC else tb["sink"]
                    qraw = rope_pool.tile([128, 512], bf16, tag="qraw")
                    nc.scalar.activation(qraw[:], ps[:], AF.Copy)
                    qsw = rope_pool.tile([128, 512], bf16, tag="qsw")
                    nc.scalar.activation(qsw[0:64, :], ps[64:128, :], AF.Copy)
                    nc.scalar.activation(qsw[64:128, :], ps[0:64, :], AF.Copy)
                    tmp = rope_pool.tile([128, 512], bf16, tag="tmp")
                    half_eng.tensor_mul(tmp[:], qsw[:], st[:, s0 : s0 + 512])
                    qc = rope_pool.tile([128, 512], bf16, tag="qc")
                    nc.vector.tensor_mul(qc[:], qraw[:], ct[:, s0 : s0 + 512])
                    add_eng.tensor_add(
                        qk_tiles[t_o][:, s0 : s0 + 512], qc[:], tmp[:]
                    )

                def emit_v_group(sc, st_i):
                    psv = psq_pool.tile([128, 512], f32, tag="psq", name="psv")
                    for k in range(NKT):
                        nc.tensor.matmul(
                            psv[:],
                            xt(k, st_i * 128, (st_i + 1) * 128),
                            wt(k, 2 * HPC * 128, 3 * HPC * 128),
                            start=(k == 0),
                            stop=(k == NKT - 1),
                        )
                    getattr(nc, cfg["qkv_evac"]).activation(
                        vsd_tiles[sc * 4 + st_i][:], psv[:], AF.Copy
                    )

                def process_block(h, i):
                    qh = qk_tiles[h]
                    kh = qk_tiles[HPC + h]
                    jlo = max(0, i * 128 - WINDOW)
                    w = i * 128 + 128 - jlo
                    mo = 640 - w
                    ps_s = pss_pool.tile([128, 640], f32, tag="pss")
                    if cfg["edge_mask"]:
                        tris = []
                        if w == 640:
                            tris.append((0, 128))
                        tris.append((w - 128, w))
                        cuts = sorted({0, w, 512} | {c for t in tris for c in t})
                        cuts = [c for c in cuts if 0 <= c <= w]
                        pieces = list(zip(cuts, cuts[1:]))

                        def score_mm(c, nxt, first):
                            nc.tensor.matmul(
                                ps_s[:, c:nxt],
                                qh[:, i * 128 : (i + 1) * 128],
                                kh[:, jlo + c : jlo + nxt],
                                start=first, stop=True,
                                skip_group_check=True,
                            )

                        for c, nxt in pieces:
                            if not any(c0 <= c < c1 for c0, c1 in tris):
                                score_mm(c, nxt, True)
                        for c0, c1 in tris:
                            nc.tensor.matmul(
                                ps_s[:, c0:c1], idnb, msk[:, mo + c0 : mo + c1],
                                start=True, stop=False, skip_group_check=True,
                            )
                        for c, nxt in pieces:
                            if any(c0 <= c < c1 for c0, c1 in tris):
                                score_mm(c, nxt, False)
                    else:
                        c = 0
                        while c < w:
                            nxt = min(w, (c // 512 + 1) * 512)
                            nc.tensor.matmul(
                                ps_s[:, c:nxt], idnb, msk[:, mo + c : mo + nxt],
                                start=True, stop=False, skip_group_check=True,
                            )
                            c = nxt
                        c = 0
                        while c < w:
                            nxt = min(w, (c // 512 + 1) * 512)
                            nc.tensor.matmul(
                                ps_s[:, c:nxt],
                                qh[:, i * 128 : (i + 1) * 128],
                                kh[:, jlo + c : jlo + nxt],
                                start=False, stop=True, skip_group_check=True,
                            )
                            c = nxt
                    pm = pm_pool.tile([128, 640], bf16, tag="pm")
                    sums = sm_pool.tile([128, 1], f32, tag="sums")
                    nc.scalar.activation(
                        pm[:, :w], ps_s[:, :w], AF.Exp, accum_out=sums[:]
                    )
                    rc = sm_pool.tile([128, 1], f32, tag="rc")
                    nc.vector.reciprocal(rc[:], sums[:])
                    pr = pr_pool.tile([128, 640], bf16, tag="pr")
                    if cfg["norm_eng"] == "vector":
                        nc.vector.tensor_scalar_mul(pr[:, :w], pm[:, :w], rc[:])
                    else:
                        nc.scalar.activation(pr[:, :w], pm[:, :w], AF.Copy, scale=rc[:])
                    nblk = w // 128
                    stg = stg_pool.tile(
                        [128, 640], bf16, tag=f"stg{h}", name=f"stg{h}_{i}_{rep}"
                    )
                    stages[h][i] = stg
                    stg3 = stg[:].rearrange("p (a c) -> p a c", c=128)
                    tr_eng.dma_start_transpose(
                        stg3[:, 0:nblk, :], pr[:, 0 : nblk * 128]
                    )

                def emit_pv(t):
                    psot = px_pool.tile(
                        [128, 512], f32, tag="px", name=f"psot{t}_{rep}"
                    )
                    jlo_t = max(0, t - 4)
                    nc.tensor.matmul(
                        psot[:], idnb, zerosb[:],
                        start=True, stop=False, skip_group_check=True,
                    )
                    for jb in range(jlo_t, t + 1):
                        z = jb - jlo_t
                        for h in range(HPC):
                            nc.tensor.matmul(
                                psot[:, h * 128 : (h + 1) * 128],
                                vsd_tiles[jb][:, h * 128 : (h + 1) * 128],
                                stages[h][t][:, z * 128 : (z + 1) * 128],
                                start=False,
                                stop=(jb == t),
                                skip_group_check=True,
                            )
                    if cfg["pso_evac"] == "vector":
                        nc.vector.tensor_copy(
                            ah_all[:, t * 512 : (t + 1) * 512], psot[:]
                        )
                    else:
                        nc.scalar.activation(
                            ah_all[:, t * 512 : (t + 1) * 512], psot[:], AF.Copy
                        )

                def emit_oproj(t):
                    ob = ob_pool.tile(
                        [128, HIDDEN], bf16, tag="ob", name=f"ob{t}_{rep}"
                    )
                    for mc in range(HIDDEN // 512):
                        ps = px_pool.tile([128, 512], f32, tag="px", name="psc")
                        for hh in range(HPC):
                            nc.tensor.matmul(
                                ps[:],
                                ah_all[
                                    :, (t * HPC + hh) * 128 : (t * HPC + hh + 1) * 128
                                ],
                                wo_all[
                                    :,
                                    hh * HIDDEN + mc * 512 : hh * HIDDEN + (mc + 1) * 512,
                                ],
                                start=(hh == 0),
                                stop=(hh == HPC - 1),
                            )
                        if cfg["c_evac_alt"] and mc % 2 == 1:
                            nc.scalar.activation(
                                ob[:, mc * 512 : (mc + 1) * 512], ps[:], AF.Copy
                            )
                        else:
                            nc.vector.tensor_copy(
                                ob[:, mc * 512 : (mc + 1) * 512], ps[:]
                            )
                    getattr(nc, cfg["out_eng"]).dma_start(
                        out_d[t * 128 : (t + 1) * 128, :], ob[:]
                    )

                def emit_segment(seg):
                    t = seg - LAG
                    to = seg - OLAG
                    if seg < NST:
                        process_block(0, seg)
                        process_block(1, seg)
                    if 0 <= t < NST:
                        emit_pv(t)
                    if seg < NST:
                        process_block(2, seg)
                        process_block(3, seg)
                    if 0 <= to < NST:
                        emit_oproj(to)

                # Interleaved schedule: chunk sc's 12 projection groups carry
                # the 4 attention segments of chunk sc-1 between them.
                GAP = cfg["gap"]
                for sc in range(NSC):
                    emit_xt(sc)
                    groups = [("qk", t_o) for t_o in range(2 * HPC)] + [
                        ("v", st_i) for st_i in range(4)
                    ]
                    segs = (
                        list(range((sc - 1) * 4, sc * 4)) if sc >= 1 else []
                    )
                    for gi, (kind, idx) in enumerate(groups):
                        if kind == "qk":
                            emit_qk_group(sc, idx)
                        else:
                            emit_v_group(sc, idx)
                        if gi % GAP == GAP - 1 and segs:
                            emit_segment(segs.pop(0))
                    for s in segs:
                        emit_segment(s)
                # remaining segments: last chunk's blocks + lag tails
                for seg in range((NSC - 1) * 4, NST + OLAG):
                    emit_segment(seg)

    nc.compile()
    return nc


def _get_module(repeat=1, cfg=None):
    key = ("nc", repeat, tuple(sorted((cfg or {}).items())))
    if key not in _CACHE:
        _CACHE[key] = _build_module(repeat, cfg)
    return _CACHE[key]


def make_in_maps(hidden_states, cos, sin, w_qkv, w_o):
    import ml_dtypes

    bf = ml_dtypes.bfloat16
    hidden_states = np.asarray(hidden_states, dtype=np.float32)
    cos = np.asarray(cos, dtype=np.float32)
    sin = np.asarray(sin, dtype=np.float32)
    w_qkv = np.asarray(w_qkv, dtype=np.float32)
    w_o = np.asarray(w_o, dtype=np.float32)

    cosT = np.ascontiguousarray(cos.T)  # [DH, S]
    sinT = np.ascontiguousarray(sin.T)
    sinS = sinT.copy()
    sinS[: DH // 2] *= -1.0  # fold rotate_half sign
    tbls = np.concatenate(
        [cosT * SCALE, sinS * SCALE, cosT, sinS], axis=1
    ).astype(bf)

    qi = np.arange(128)[:, None]
    jj = np.arange(640)[None, :]
    mask = np.where((jj > qi) & (jj <= qi + WINDOW), 0.0, NEG).astype(np.float32)
    cmask = np.concatenate([mask, np.eye(128, dtype=np.float32)], axis=1).astype(bf)

    xTs = [np.ascontiguousarray(hidden_states[b].T).astype(bf) for b in range(B)]

    in_maps = []
    for c in range(N_CORES):
        b, hg = divmod(c, N_CORES // B)
        r0 = hg * HPC * DH
        wq = w_qkv[r0 : r0 + HPC * DH]
        wk = w_qkv[N_HEADS * DH + r0 : N_HEADS * DH + r0 + HPC * DH]
        wv = w_qkv[2 * N_HEADS * DH + r0 : 2 * N_HEADS * DH + r0 + HPC * DH]
        wTc = np.ascontiguousarray(np.concatenate([wq, wk, wv], axis=0).T).astype(bf)
        woT = w_o[:, r0 : r0 + HPC * DH].T  # [512, HIDDEN]
        wo_cat = np.concatenate(
            [woT[hh * 128 : (hh + 1) * 128] for hh in range(HPC)], axis=1
        ).astype(bf)
        in_maps.append(
            {
                "xT": xTs[b],
                "wT": wTc,
                "wo": np.ascontiguousarray(wo_cat),
                "tbls": tbls,
                "cmask": cmask,
            }
        )
    return in_maps


def gather(results):
    out = np.zeros((B, S, HIDDEN), dtype=np.float32)
    for c in range(N_CORES):
        b = c // (N_CORES // B)
        out[b] += results[c]["out"].astype(np.float32)
    return out


def kernel(hidden_states, cos, sin, w_qkv, w_o):
    from concourse.bass_utils import run_bass_kernel_spmd

    nc = _get_module()
    in_maps = make_in_maps(hidden_states, cos, sin, w_qkv, w_o)
    res = run_bass_kernel_spmd(nc, in_maps, list(range(N_CORES)))
    return gather(res.results)


# revision 11
# speedup vs baseline: 1.3907x; 1.0048x over previous
"""Sliding-window attention v4: full A<->B+C interleave.

v4 vs v3: the attention+o_proj segments for sequence chunk sc-1 are emitted
between the QKV projection groups of chunk sc, so every cross-engine latency
chain (scores -> exp -> norm -> transpose -> PV -> evac -> o_proj) hides under
the dense QKV matmul stream instead of stalling the in-order PE queue.

PSUM budget (8 banks): psq ring shared by q/k/v projection groups (2), score
psum [128,640] x2 (4), one ring shared by PV-out and o_proj accumulators (2).
"""
import sys

sys.path.insert(0, "/opt/trn_rl_repo")

import numpy as np

B = 2
S = 2048
HIDDEN = 2048
N_HEADS = 16
DH = 128
WINDOW = 512
HPC = 4  # heads per core
N_CORES = 8
QKV_O = 3 * HPC * DH  # 1536
SCALE = 1.0 / np.sqrt(DH)
NEG = -1.0e30

_CACHE = {}

CFG = {
    "psq_bufs": 2, "pss_bufs": 2, "px_bufs": 2,
    "xt_bufs": 5, "rope_bufs": 2,
    "pm_bufs": 3, "pr_bufs": 3, "sm_bufs": 8,
    "ob_bufs": 2, "stg_bufs": 2,
    "qkv_evac": "scalar",
    "rope_half_eng": "gpsimd", "rope_add_eng": "vector",
    "norm_eng": "vector",
    "pso_evac": "vector",
    "c_evac_alt": True,
    "edge_mask": True,
    "lag": 1,    # segments between scores(t) and PV(t)
    "olag": 2,   # segments between scores(t) and o_proj(t)
    "tr_eng": "sync", "xt_eng": "sync", "out_eng": "sync",
    "wt_chunks": 6,
    "gap": 3,    # A-groups between consecutive segments in the interleave
}


def _build_module(repeat=1, cfg=None):
    cfg = {**CFG, **(cfg or {})}
    import concourse.tile as tile
    from concourse import bacc, mybir
    from contextlib import ExitStack

    f32 = mybir.dt.float32
    bf16 = mybir.dt.bfloat16
    AF = mybir.ActivationFunctionType

    nc = bacc.Bacc("TRN2", target_bir_lowering=False, debug=False)

    xT = nc.declare_dram_parameter("xT", [HIDDEN, S], bf16, isOutput=False)
    wT = nc.declare_dram_parameter("wT", [HIDDEN, QKV_O], bf16, isOutput=False)
    wo_d = nc.declare_dram_parameter("wo", [128, HPC * HIDDEN], bf16, isOutput=False)
    tbl_d = nc.declare_dram_parameter("tbls", [DH, 4 * S], bf16, isOutput=False)
    cm_d = nc.declare_dram_parameter("cmask", [128, 768], bf16, isOutput=False)
    out_d = nc.declare_dram_parameter("out", [S, HIDDEN], bf16, isOutput=True)

    NKT = HIDDEN // 128  # 16 contraction chunks
    NSC = S // 512  # 4 sequence chunks
    NST = S // 128  # 16 sequence tiles
    LAG = cfg["lag"]
    OLAG = cfg["olag"]
    WTC = cfg["wt_chunks"]

    with tile.TileContext(nc) as tc, ExitStack() as top:
        cpool = top.enter_context(tc.tile_pool(name="consts", bufs=1))
        cm = cpool.tile([128, 768], bf16, tag="cmask")
        msk = cm[:, 0:640]
        idnb = cm[:, 640:768]
        wo_all = cpool.tile([128, HPC * HIDDEN], bf16, tag="wo")
        tbl = cpool.tile([DH, 4 * S], bf16, tag="tbls")
        wt_all = cpool.tile([128, NKT * QKV_O], bf16, tag="wt")
        zerosb = cpool.tile([128, 512], bf16, tag="zerosb")
        nc.vector.memset(zerosb[:], 0)
        tb = {
            "cosq": tbl[:, 0 * S : 1 * S],
            "sinq": tbl[:, 1 * S : 2 * S],
            "cosk": tbl[:, 2 * S : 3 * S],
            "sink": tbl[:, 3 * S : 4 * S],
        }

        def wt(k, c0, c1):
            return wt_all[:, k * QKV_O + c0 : k * QKV_O + c1]

        qk_pool = top.enter_context(tc.tile_pool(name="qk", bufs=1))
        vsd_pool = top.enter_context(tc.tile_pool(name="vsd", bufs=NST))
        ah_pool = top.enter_context(tc.tile_pool(name="ah", bufs=1))

        wv = wT[:].rearrange("(k p) c -> p k c", p=128)
        xv = xT[:].rearrange("(k p) s -> p k s", p=128)

        for rep in range(repeat):
            qk_tiles = [
                qk_pool.tile([128, S], bf16, tag=f"qk{t_o}", name=f"qk{t_o}_{rep}")
                for t_o in range(2 * HPC)
            ]
            vsd_tiles = [
                vsd_pool.tile([128, HPC * DH], bf16, tag="vsd", name=f"vsd{st}_{rep}")
                for st in range(NST)
            ]
            ah_all = ah_pool.tile([128, NST * 512], bf16, tag="ah", name=f"ah_{rep}")

            with ExitStack() as ph:
                xt_pool = ph.enter_context(
                    tc.tile_pool(name="xt", bufs=cfg["xt_bufs"])
                )
                rope_pool = ph.enter_context(
                    tc.tile_pool(name="rope", bufs=cfg["rope_bufs"])
                )
                pm_pool = ph.enter_context(tc.tile_pool(name="pm", bufs=cfg["pm_bufs"]))
                pr_pool = ph.enter_context(tc.tile_pool(name="pr", bufs=cfg["pr_bufs"]))
                sm_pool = ph.enter_context(tc.tile_pool(name="sm", bufs=cfg["sm_bufs"]))
                stg_pool = ph.enter_context(
                    tc.tile_pool(name="stg", bufs=cfg["stg_bufs"])
                )
                ob_pool = ph.enter_context(tc.tile_pool(name="ob", bufs=cfg["ob_bufs"]))
                psq_pool = ph.enter_context(
                    tc.tile_pool(name="psq", bufs=cfg["psq_bufs"], space="PSUM")
                )
                pss_pool = ph.enter_context(
                    tc.tile_pool(name="pss", bufs=cfg["pss_bufs"], space="PSUM")
                )
                px_pool = ph.enter_context(
                    tc.tile_pool(name="px", bufs=cfg["px_bufs"], space="PSUM")
                )

                half_eng = getattr(nc, cfg["rope_half_eng"])
                add_eng = getattr(nc, cfg["rope_add_eng"])
                tr_eng = getattr(nc, cfg["tr_eng"])
                stages = [[None] * NST for _ in range(HPC)]
                xt_cur = []  # current chunk's x tiles

                def emit_xt(sc):
                    s0 = sc * 512
                    del xt_cur[:]
                    # wt is chunked by OUTPUT columns (projection groups), not
                    # k-rows: group t_o only needs chunk t_o*128//cpc, so the
                    # first matmul group completes after ~5.6MB of DMA (xt sc0
                    # + tbl + chunk0) instead of the full 10.3MB. tbl goes
                    # early because the rope muls gate the rope/psq rings.
                    wtv = wt_all[:].rearrange("p (k c) -> p k c", c=QKV_O)
                    cpc = QKV_O // WTC

                    def wt_chunk(ci):
                        nc.sync.dma_start(
                            wtv[:, :, ci * cpc : (ci + 1) * cpc],
                            wv[:, :, ci * cpc : (ci + 1) * cpc],
                        )

                    for qf in range(4):
                        t = xt_pool.tile([128, 4 * 512], bf16, tag="xt")
                        getattr(nc, cfg["xt_eng"]).dma_start(
                            t[:], xv[:, qf * 4 : qf * 4 + 4, s0 : s0 + 512]
                        )
                        xt_cur.append(t)
                        if rep == 0 and sc == 0:
                            if qf == 0:
                                wt_chunk(0)
                            elif qf == 1:
                                nc.sync.dma_start(tbl[:], tbl_d[:])
                    if rep == 0 and sc == 0:
                        for ci in range(1, WTC):
                            wt_chunk(ci)
                        nc.sync.dma_start(cm[:], cm_d[:])
                        nc.sync.dma_start(wo_all[:], wo_d[:])

                def xt(k, c0=0, c1=512):
                    return xt_cur[k // 4][:, (k % 4) * 512 + c0 : (k % 4) * 512 + c1]

                def emit_qk_group(sc, t_o):
                    s0 = sc * 512
                    ps = psq_pool.tile([128, 512], f32, tag="psq")
                    for k in range(NKT):
                        nc.tensor.matmul(
                            ps[:],
                            wt(k, t_o * 128, (t_o + 1) * 128),
                            xt(k),
                            start=(k == 0),
                            stop=(k == NKT - 1),
                        )
                    ct = tb["cosq"] if t_o < HPC else tb["cosk"]
                    st = tb["sinq"] if t_o < HPC else tb["sink"]
                    qraw = rope_pool.tile([128, 512], bf16, tag="qraw")
                    nc.scalar.activation(qraw[:], ps[:], AF.Copy)
                    qsw = rope_pool.tile([128, 512], bf16, tag="qsw")
                    nc.scalar.activation(qsw[0:64, :], ps[64:128, :], AF.Copy)
                    nc.scalar.activation(qsw[64:128, :], ps[0:64, :], AF.Copy)
                    tmp = rope_pool.tile([128, 512], bf16, tag="tmp")
                    half_eng.tensor_mul(tmp[:], qsw[:], st[:, s0 : s0 + 512])
                    qc = rope_pool.tile([128, 512], bf16, tag="qc")
                    nc.vector.tensor_mul(qc[:], qraw[:], ct[:, s0 : s0 + 512])
                    add_eng.tensor_add(
                        qk_tiles[t_o][:, s0 : s0 + 512], qc[:], tmp[:]
                    )

                def emit_v_group(sc, st_i):
                    psv = psq_pool.tile([128, 512], f32, tag="psq", name="psv")
                    for k in range(NKT):
                        nc.tensor.matmul(
                            psv[:],
                            xt(k, st_i * 128, (st_i + 1) * 128),
                            wt(k, 2 * HPC * 128, 3 * HPC * 128),
                            start=(k == 0),
                            stop=(k == NKT - 1),
                        )
                    getattr(nc, cfg["qkv_evac"]).activation(
                        vsd_tiles[sc * 4 + st_i][:], psv[:], AF.Copy
                    )

                def process_block(h, i):
                    qh = qk_tiles[h]
                    kh = qk_tiles[HPC + h]
                    jlo = max(0, i * 128 - WINDOW)
                    w = i * 128 + 128 - jlo
                    mo = 640 - w
                    ps_s = pss_pool.tile([128, 640], f32, tag="pss")
                    if cfg["edge_mask"]:
                        tris = []
                        if w == 640:
                            tris.append((0, 128))
                        tris.append((w - 128, w))
                        cuts = sorted({0, w, 512} | {c for t in tris for c in t})
                        cuts = [c for c in cuts if 0 <= c <= w]
                        pieces = list(zip(cuts, cuts[1:]))

                        def score_mm(c, nxt, first):
                            nc.tensor.matmul(
                                ps_s[:, c:nxt],
                                qh[:, i * 128 : (i + 1) * 128],
                                kh[:, jlo + c : jlo + nxt],
                                start=first, stop=True,
                                skip_group_check=True,
                            )

                        for c, nxt in pieces:
                            if not any(c0 <= c < c1 for c0, c1 in tris):
                                score_mm(c, nxt, True)
                        for c0, c1 in tris:
                            nc.tensor.matmul(
                                ps_s[:, c0:c1], idnb, msk[:, mo + c0 : mo + c1],
                                start=True, stop=False, skip_group_check=True,
                            )
                        for c, nxt in pieces:
                            if any(c0 <= c < c1 for c0, c1 in tris):
                                score_mm(c, nxt, False)
                    else:
                        c = 0
                        while c < w:
                            nxt = min(w, (c // 512 + 1) * 512)
                            nc.tensor.matmul(
                                ps_s[:, c:nxt], idnb, msk[:, mo + c : mo + nxt],
                                start=True, stop=False, skip_group_check=True,
                            )
                            c = nxt
                        c = 0
                        while c < w:
                            nxt = min(w, (c // 512 + 1) * 512)
                            nc.tensor.matmul(
                                ps_s[:, c:nxt],
                                qh[:, i * 128 : (i + 1) * 128],
                                kh[:, jlo + c : jlo + nxt],
                                start=False, stop=True, skip_group_check=True,
                            )
                            c = nxt
                    pm = pm_pool.tile([128, 640], bf16, tag="pm")
                    sums = sm_pool.tile([128, 1], f32, tag="sums")
                    nc.scalar.activation(
                        pm[:, :w], ps_s[:, :w], AF.Exp, accum_out=sums[:]
                    )
                    rc = sm_pool.tile([128, 1], f32, tag="rc")
                    nc.vector.reciprocal(rc[:], sums[:])
                    pr = pr_pool.tile([128, 640], bf16, tag="pr")
                    if cfg["norm_eng"] == "vector":
                        nc.vector.tensor_scalar_mul(pr[:, :w], pm[:, :w], rc[:])
                    else:
                        nc.scalar.activation(pr[:, :w], pm[:, :w], AF.Copy, scale=rc[:])
                    nblk = w // 128
                    stg = stg_pool.tile(
                        [128, 640], bf16, tag=f"stg{h}", name=f"stg{h}_{i}_{rep}"
                    )
                    stages[h][i] = stg
                    stg3 = stg[:].rearrange("p (a c) -> p a c", c=128)
                    tr_eng.dma_start_transpose(
                        stg3[:, 0:nblk, :], pr[:, 0 : nblk * 128]
                    )

                def emit_pv(t):
                    psot = px_pool.tile(
                        [128, 512], f32, tag="px", name=f"psot{t}_{rep}"
                    )
                    jlo_t = max(0, t - 4)
                    nc.tensor.matmul(
                        psot[:], idnb, zerosb[:],
                        start=True, stop=False, skip_group_check=True,
                    )
                    for jb in range(jlo_t, t + 1):
                        z = jb - jlo_t
                        for h in range(HPC):
                            nc.tensor.matmul(
                                psot[:, h * 128 : (h + 1) * 128],
                                vsd_tiles[jb][:, h * 128 : (h + 1) * 128],
                                stages[h][t][:, z * 128 : (z + 1) * 128],
                                start=False,
                                stop=(jb == t),
                                skip_group_check=True,
                            )
                    if cfg["pso_evac"] == "vector":
                        nc.vector.tensor_copy(
                            ah_all[:, t * 512 : (t + 1) * 512], psot[:]
                        )
                    else:
                        nc.scalar.activation(
                            ah_all[:, t * 512 : (t + 1) * 512], psot[:], AF.Copy
                        )

                def emit_oproj(t):
                    ob = ob_pool.tile(
                        [128, HIDDEN], bf16, tag="ob", name=f"ob{t}_{rep}"
                    )
                    for mc in range(HIDDEN // 512):
                        ps = px_pool.tile([128, 512], f32, tag="px", name="psc")
                        for hh in range(HPC):
                            nc.tensor.matmul(
                                ps[:],
                                ah_all[
                                    :, (t * HPC + hh) * 128 : (t * HPC + hh + 1) * 128
                                ],
                                wo_all[
                                    :,
                                    hh * HIDDEN + mc * 512 : hh * HIDDEN + (mc + 1) * 512,
                                ],
                                start=(hh == 0),
                                stop=(hh == HPC - 1),
                            )
                        if cfg["c_evac_alt"] and mc % 2 == 1:
                            nc.scalar.activation(
                                ob[:, mc * 512 : (mc + 1) * 512], ps[:], AF.Copy
                            )
                        else:
                            nc.vector.tensor_copy(
                                ob[:, mc * 512 : (mc + 1) * 512], ps[:]
                            )
                    getattr(nc, cfg["out_eng"]).dma_start(
                        out_d[t * 128 : (t + 1) * 128, :], ob[:]
                    )

                def emit_segment(seg):
                    t = seg - LAG
                    to = seg - OLAG
                    if seg < NST:
                        process_block(0, seg)
                        process_block(1, seg)
                    if 0 <= t < NST:
                        emit_pv(t)
                    if seg < NST:
                        process_block(2, seg)
                        process_block(3, seg)
                    if 0 <= to < NST:
                        emit_oproj(to)

                # Interleaved schedule: chunk sc's 12 projection groups carry
                # the 4 attention segments of chunk sc-1 between them.
                GAP = cfg["gap"]
                for sc in range(NSC):
                    emit_xt(sc)
                    groups = [("qk", t_o) for t_o in range(2 * HPC)] + [
                        ("v", st_i) for st_i in range(4)
                    ]
                    segs = (
                        list(range((sc - 1) * 4, sc * 4)) if sc >= 1 else []
                    )
                    for gi, (kind, idx) in enumerate(groups):
                        if kind == "qk":
                            emit_qk_group(sc, idx)
                        else:
                            emit_v_group(sc, idx)
                        if gi % GAP == GAP - 1 and segs:
                            emit_segment(segs.pop(0))
                    for s in segs:
                        emit_segment(s)
                # remaining segments: last chunk's blocks + lag tails
                for seg in range((NSC - 1) * 4, NST + OLAG):
                    emit_segment(seg)

    nc.compile()
    return nc


def _get_module(repeat=1, cfg=None):
    key = ("nc", repeat, tuple(sorted((cfg or {}).items())))
    if key not in _CACHE:
        _CACHE[key] = _build_module(repeat, cfg)
    return _CACHE[key]


def make_in_maps(hidden_states, cos, sin, w_qkv, w_o):
    import ml_dtypes

    bf = ml_dtypes.bfloat16
    hidden_states = np.asarray(hidden_states, dtype=np.float32)
    cos = np.asarray(cos, dtype=np.float32)
    sin = np.asarray(sin, dtype=np.float32)
    w_qkv = np.asarray(w_qkv, dtype=np.float32)
    w_o = np.asarray(w_o, dtype=np.float32)

    cosT = np.ascontiguousarray(cos.T)  # [DH, S]
    sinT = np.ascontiguousarray(sin.T)
    sinS = sinT.copy()
    sinS[: DH // 2] *= -1.0  # fold rotate_half sign
    tbls = np.concatenate(
        [cosT * SCALE, sinS * SCALE, cosT, sinS], axis=1
    ).astype(bf)

    qi = np.arange(128)[:, None]
    jj = np.arange(640)[None, :]
    mask = np.where((jj > qi) & (jj <= qi + WINDOW), 0.0, NEG).astype(np.float32)
    cmask = np.concatenate([mask, np.eye(128, dtype=np.float32)], axis=1).astype(bf)

    xTs = [np.ascontiguousarray(hidden_states[b].T).astype(bf) for b in range(B)]

    in_maps = []
    for c in range(N_CORES):
        b, hg = divmod(c, N_CORES // B)
        r0 = hg * HPC * DH
        wq = w_qkv[r0 : r0 + HPC * DH]
        wk = w_qkv[N_HEADS * DH + r0 : N_HEADS * DH + r0 + HPC * DH]
        wv = w_qkv[2 * N_HEADS * DH + r0 : 2 * N_HEADS * DH + r0 + HPC * DH]
        wTc = np.ascontiguousarray(np.concatenate([wq, wk, wv], axis=0).T).astype(bf)
        woT = w_o[:, r0 : r0 + HPC * DH].T  # [512, HIDDEN]
        wo_cat = np.concatenate(
            [woT[hh * 128 : (hh + 1) * 128] for hh in range(HPC)], axis=1
        ).astype(bf)
        in_maps.append(
            {
                "xT": xTs[b],
                "wT": wTc,
                "wo": np.ascontiguousarray(wo_cat),
                "tbls": tbls,
                "cmask": cmask,
            }
        )
    return in_maps


def gather(results):
    out = np.zeros((B, S, HIDDEN), dtype=np.float32)
    for c in range(N_CORES):
        b = c // (N_CORES // B)
        out[b] += results[c]["out"].astype(np.float32)
    return out


def kernel(hidden_states, cos, sin, w_qkv, w_o):
    from concourse.bass_utils import run_bass_kernel_spmd

    nc = _get_module()
    in_maps = make_in_maps(hidden_states, cos, sin, w_qkv, w_o)
    res = run_bass_kernel_spmd(nc, in_maps, list(range(N_CORES)))
    return gather(res.results)
